# revision 68
# baseline (speedup 1.0000x reference)
"""Trainium2 Bass kernel for nn_MetaNet (triu-gram features -> Wh matvec ->
relu -> 14 per-head linears), distributed over 8 NeuronCores.

v4: fp8e3 weights, NO collective, split-path stage 1.

Sharding: the 8 cores launch with tens of us of skew, so any cross-core
dependency (AllGather) makes every core wait for the slowest straggler.
Stage 1 is sharded by WHOLE heads: cores 0-5 own 2 heads (256 hidden
rows), cores 6-7 own 1 head + a zero-padded slot (uniform SPMD
program). Stage 2 needs only the core's own hidden -> everything local.

Quantization: Wh/Wf host-quantized to float8e3 (e3m4) with exact-range
scales SH=15.5/absmax(Wh), SF=15.5/absmax(Wf). The feature image and
the g4 tensor are host-prescaled by 1/(SH*SF) (the device pool constant
stays fixed), so hidden comes out scaled by 1/SF and stage-2
(SF*Wf)(hidden/SF) is exact. Mixed-dtype matmuls (bf16 x fp8) verified
bit-exact on HW including fp8 subnormals.

Stage 1 weights-stationary: per contraction chunk each slot's 128x128
fp8 weight block is the STATIONARY operand (LDWEIGHTS+FWL pipelines
under neighbouring matmuls - measured 34ns per LDW+MM(N=1) pair in
stage 2) and the feat column is the moving operand; psum [128,1] per
slot accumulates in column form - exactly what stage 2 wants, no
transpose. ~68ns/chunk PE vs 109ns with the weights-moving orientation,
which makes stage 1 DMA-bound (91.5ns/chunk at 358GB/s). A v4
experiment that alternated wh-stationary and feat-stationary matmuls
serialized LDWEIGHTS (313ns/chunk) - never alternate stationary
sources.

Layout:
  feat:    packed 76128-dim feature on a [128, 632] bf16 grid split into
           featB (cols 576+, host-image only, ready first) and featA
           (cols 0..576). triu(g1,g2,g3) is host-packed into the DMA
           image; g4 avg-pooled on device, moved as 8 rectangles whose
           below-diagonal garbage lands on host-zeroed Wh columns.
  stage 1: 632 chunks, 2 matmuls each (see above); wh streamed in 17
           DMAs, first block small so matmuls start ~11us.
  wf:      fp8e3, 4 block DMAs issued on the sync queue right after the
           wh blocks (FIFO gives wh priority; no_sync_barrier stops the
           scheduler hoisting them) -> wf stream overlaps stage-1.
  stage 2: 576 matmuls ([128,128] fp8 wf stationary + hstat column
           moving); 8 psum slices -> copies -> out DMAs overlap the
           tail.
  host:    head slots mapped back to the 14 global heads; bf added.
"""

import math
from dataclasses import dataclass

import numpy as np
import ml_dtypes

BF = ml_dtypes.bfloat16
F8 = ml_dtypes.float8_e3m4

F8MAX = 15.5
POOLC = 0.25 * 2.0**-15   # fixed device pool constant; host prescales g4

# core -> (head slot0, head slot1); -1 = zero-padded slot
HEAD_MAP = [(0, 1), (2, 3), (4, 5), (6, 7), (8, 9), (10, 11),
            (12, -1), (13, -1)]


@dataclass(frozen=True)
class Cfg:
    n_cores: int = 8
    cs: tuple = (64, 128, 256, 256)   # g1, g2, g3, pooled g4
    hid: int = 128
    nl: int = 14
    d2: int = 36864
    nk: int = 632                     # feat grid columns
    slots: int = 2                    # head slots per core

    @property
    def tri123(self):
        return sum(c * (c + 1) // 2 for c in self.cs[:3])  # 43232

    @property
    def hidden(self):
        return self.hid * self.nl                          # 1792

    @property
    def rows(self):
        return self.hid * self.slots                       # 256 rows/core

    @property
    def nblk(self):
        return self.d2 // 128                              # 288 col-blocks/head

    @property
    def t2(self):
        return self.slots * self.nblk                      # 576 stage-2 tiles


FULL = Cfg()


# pooled-g4 rectangle placement: 8 blocks of 32 pooled rows each, copied as
# whole [32, 256-r0] rectangles (below-diagonal cells map to zero Wh columns).
RECTS = [  # (r0, P, C, W)
    (0,   64, 0,   256), (32,  64, 256, 224), (192, 64, 480, 64),
    (224, 64, 544, 32),
    (64,  96, 0,   192), (96,  96, 192, 160), (128, 96, 352, 128),
    (160, 96, 480, 96),
]

# contraction chunk order: grid columns 576+ (tri123-image only, no rectangle
# dependency) first, so stage-1 matmuls can start before the pooled-g4
# rectangles land in SBUF. The host lays Wh blocks out in the same order.
PERM = list(range(576, 632)) + list(range(576))


def build_nc(cfg: Cfg):
    import concourse.bacc as bacc
    import concourse.tile as tile
    import concourse.mybir as mybir

    f32 = mybir.dt.float32
    bf16 = mybir.dt.bfloat16
    f8 = mybir.dt.float8e3
    nc = bacc.Bacc("TRN2", target_bir_lowering=False, debug=False,
                   num_devices=cfg.n_cores)

    nk, rows, t2, hid = cfg.nk, cfg.rows, cfg.t2, cfg.hid
    # wf blocks at ~1.2MB (the wh stream's measured density sweet spot),
    # with a tiny last block so the final matmuls + out chain ride a
    # short tail after the stream's last bytes
    wf_chunks = [72] * 7 + [60, 12]
    assert sum(wf_chunks) == t2
    wf_starts = [sum(wf_chunks[:b]) for b in range(len(wf_chunks))]
    blk_of = []
    for b, w in enumerate(wf_chunks):
        blk_of += [b] * w

    g4 = nc.dram_tensor("g4", [128, 2048], bf16, kind="ExternalInput")
    img = nc.dram_tensor("img", [128, nk], bf16, kind="ExternalInput")
    wh = nc.dram_tensor("wh", [128, nk * rows], f8, kind="ExternalInput")
    wf = nc.dram_tensor("wf", [128, t2 * 128], f8, kind="ExternalInput")
    bh_c = nc.dram_tensor("bh_c", [hid, cfg.slots], f32, kind="ExternalInput")
    out = nc.dram_tensor("out", [128, t2], f32, kind="ExternalOutput")

    # small first block so stage-1 matmuls start as soon as possible;
    # 40-chunk blocks measured faster than 80-chunk ones (finer sem
    # granularity keeps the stream dense).
    wh_chunks = [12] + [40] * 15 + [20]
    assert sum(wh_chunks) == nk

    with tile.TileContext(nc) as tc:
        with (
            tc.tile_pool(name="const", bufs=1) as const,
            tc.tile_pool(name="poolq", bufs=1) as poolq,
            tc.tile_pool(name="whp", bufs=6) as whp,
            tc.tile_pool(name="wfp", bufs=9) as wfp,
            tc.tile_pool(name="ps1", bufs=1, space="PSUM") as ps1p,
            tc.tile_pool(name="psw", bufs=1, space="PSUM") as pswp,
            tc.tile_pool(name="ps2", bufs=4, space="PSUM") as ps2p,
        ):
            # PE warm-up: matmuls on a memset tile keep the PE busy during
            # the initial DMA window so HAM un-throttles before stage 1.
            wmt = const.tile([128, 128], bf16)
            nc.vector.memset(wmt[:], 0.0)
            pswu = pswp.tile([1, 128], f32)
            for _ in range(30):
                nc.tensor.matmul(pswu[:], wmt[:, 0:1], wmt[:],
                                 start=True, stop=True)

            # feat grid in two tiles: featB (img-only columns, ready first)
            # and featA (columns 0..576, overwritten by pooled-g4 rects).
            # Scalar-queue order: featB, g4 (pooling is on the critical
            # path for featA), featA, biases, then the rectangles.
            # Small inputs ride the scalar ring (their ~0.65us per-DMA
            # issue cost overlaps the sync ring's issues); the 1MB g4 load
            # rides the sync ring AHEAD of the wh stream as 2 merged DMAs
            # (row-pair interleave folded into the access pattern) because
            # the scalar ring is starved once the sync ring streams and
            # the g4 -> pool -> rect chain gates the featA chunks.
            featB = const.tile([128, nk - 576], bf16)
            nc.scalar.dma_start(featB[:], img[:, 576:nk])
            featA = const.tile([128, 576], bf16)
            nc.scalar.dma_start(featA[:], img[:, 0:576])
            bh_ct = const.tile([hid, cfg.slots], f32)
            nc.scalar.dma_start(bh_ct[:], bh_c[:])

            # single g4 DMA: host pre-interleaves so partition r carries
            # rows {2r,2r+1} (half 0) then {256+2r,257+2r} (half 1) - one
            # issue slot + one completion-sem lane at the stream head
            gx = poolq.tile([128, 2048], bf16)
            nc.sync.dma_start(gx[:], g4[:])
            # keep the wh stream behind the g4 load on the sync ring
            tc.no_sync_barrier()

            # ---- avgpool g4 -> f32 sums [128,256] x2 on device ----
            pooled = []
            for h in range(2):
                rs = poolq.tile([128, 512], f32, tag=f"rs{h}")
                nc.vector.tensor_add(rs[:], gx[:, h * 1024:h * 1024 + 512],
                                     gx[:, h * 1024 + 512:(h + 1) * 1024])
                cp = poolq.tile([128, 256], f32, tag=f"cp{h}")
                nc.vector.tensor_add(cp[:], rs[:, 0::2], rs[:, 1::2])
                pooled.append(cp)

            # ---- 8 rectangle placements: scale+cast+partition-remap in one
            # ACT op each, straight into the feat grid. Keeping these off
            # the DMA path matters: DMA completion-semaphore lanes are
            # shared round-robin across queues, and late-completing rect
            # DMAs were gating wh-stream block issues for ~17us.
            for ri, (r0, P, C, W) in enumerate(RECTS):
                srct = pooled[r0 // 128]
                a = r0 % 128
                eng = nc.scalar if ri % 2 == 0 else nc.vector
                if eng is nc.scalar:
                    nc.scalar.mul(featA[P:P + 32, C:C + W],
                                  srct[a:a + 32, r0:r0 + W], POOLC)
                else:
                    nc.vector.tensor_scalar_mul(
                        featA[P:P + 32, C:C + W],
                        srct[a:a + 32, r0:r0 + W], POOLC)

            # ---- stage 1: weights-stationary over 632 chunks ----
            # per chunk, per slot: LDW [128,128] fp8 block + MM(feat col)
            # -> psum[slot] [128, 1] accumulates in column form
            psum_s = [ps1p.tile([hid, 1], f32, tag=f"st{s}",
                                name=f"psum_s{s}")
                      for s in range(cfg.slots)]
            k0 = 0
            for b, nch in enumerate(wh_chunks):
                whb = whp.tile([128, 40 * rows], f8, tag="whb")
                nc.sync.dma_start(
                    whb[:, :nch * rows],
                    wh[:, k0 * rows:(k0 + nch) * rows])
                for j in range(nch):
                    k = k0 + j
                    kc = PERM[k]
                    fcol = (featB[:, kc - 576:kc - 575] if kc >= 576
                            else featA[:, kc:kc + 1])
                    st = (k == 0)
                    sp = (k == nk - 1)
                    for s in range(cfg.slots):
                        nc.tensor.matmul(
                            psum_s[s][:],
                            whb[:, j * rows + s * hid:
                                j * rows + (s + 1) * hid],
                            fcol, start=st, stop=sp)
                k0 += nch

            # wf stream: issued on the sync queue AFTER the wh blocks; the
            # queue is FIFO so wh keeps strict DMA priority, but the wf
            # stream still overlaps stage-1 compute. The no_sync_barrier
            # stops the scheduler hoisting these above the wh dma_starts.
            tc.no_sync_barrier()
            wf_tiles = []   # (tile, first tile index) per block
            wt0 = 0
            for b, wnt in enumerate(wf_chunks):
                wfb = wfp.tile([128, 72 * 128], f8, tag="wfb")
                nc.sync.dma_start(
                    wfb[:, :wnt * 128],
                    wf[:, wt0 * 128:(wt0 + wnt) * 128])
                wf_tiles.append((wfb, wt0))
                wt0 += wnt

            # ---- local hidden -> hstat [128, slots] (no collective) ----
            # both slot psums are already column-oriented
            hstat = const.tile([hid, cfg.slots], bf16)
            hf = const.tile([hid, cfg.slots], f32)
            for s in range(cfg.slots):
                nc.vector.tensor_add(hf[:, s:s + 1], psum_s[s][:],
                                     bh_ct[:, s:s + 1])
            nc.vector.tensor_scalar_max(hstat[:], hf[:], 0.0)

            # ---- stage 2: 576 wf-stationary matmuls, nine psum slices.
            # The final slice is small: its matmuls+copy+out-DMA run after
            # the last wf bytes land, so it IS the kernel tail.
            slices = [72] * 7 + [48, 24]
            t0s = 0
            for hx, scnt in enumerate(slices):
                ps2 = ps2p.tile([128, 72], f32, tag="ps2")
                for u in range(scnt):
                    t = t0s + u
                    s = t // cfg.nblk
                    bi = blk_of[t]
                    wfb, wt0b = wf_tiles[bi]
                    jj = t - wt0b
                    nc.tensor.matmul(ps2[:, u:u + 1],
                                     wfb[:, jj * 128:(jj + 1) * 128],
                                     hstat[:, s:s + 1], start=True, stop=True)
                osb = const.tile([128, 72], f32, tag=f"osb{hx}")
                nc.vector.tensor_copy(osb[:, :scnt], ps2[:, :scnt])
                nc.sync.dma_start(out[:, t0s:t0s + scnt], osb[:, :scnt])
                t0s += scnt

    nc.compile()
    return nc


def _to_bf16(a: np.ndarray) -> np.ndarray:
    return np.ascontiguousarray(a.astype(BF))


def _to_f8(a: np.ndarray, scale: float) -> np.ndarray:
    return np.clip(a * scale, -F8MAX, F8MAX).astype(F8)


def shard_inputs(cfg: Cfg, g1, g2, g3, g4, Wh, bh, Wf, bf):
    """Full inputs -> list of per-core in_maps (numpy, contiguous)."""
    f32 = np.float32
    nk, rows, t2, hid = cfg.nk, cfg.rows, cfg.t2, cfg.hid
    c1, c2, c3, c4 = cfg.cs

    # exact-range fp8 scales (host-side; all descaling folds into the
    # host prescale of the feature inputs, device program is unchanged)
    SHs = float(F8MAX * (1.0 - 1e-3) / np.abs(Wh).max())
    SFs = float(F8MAX * (1.0 - 1e-3) / np.abs(Wf).max())
    fsc = 1.0 / (SHs * SFs)

    g1 = g1.reshape(c1, c1)
    g2 = g2.reshape(c2, c2)
    g3 = g3.reshape(c3, c3)
    # device multiplies pooled sums by POOLC = 0.25*2^-15; prescale g4 so
    # the net factor is 0.25*fsc. Shipped bf16: halves the 1MB load that
    # sits ahead of the wh stream on the sync ring.
    g4 = np.ascontiguousarray(
        (g4.reshape(512, 512) * (fsc * 2.0**15)).astype(BF)
        .reshape(2, 128, 2, 512).transpose(1, 0, 2, 3).reshape(128, 2048))
    bh = np.asarray(bh).reshape(cfg.hidden) * (1.0 / SFs)

    # packed triu(g1,g2,g3) -> feat grid image [128, nk] bf16, pre-scaled
    tri = np.concatenate([
        g[np.triu_indices(c)] for g, c in ((g1, c1), (g2, c2), (g3, c3))
    ]).astype(f32) * fsc
    assert tri.size == cfg.tri123
    img = np.zeros(128 * nk, dtype=f32)
    tri_slots = np.zeros((128, nk), dtype=bool)
    tri_slots[0:64, :] = True
    tri_slots[64:128, 576:] = True
    idx = np.flatnonzero(tri_slots.reshape(-1))[:cfg.tri123]
    img[idx] = tri
    img = _to_bf16(img.reshape(128, nk))

    # column map: grid slot (p,k) -> Wh column (or -1 for padding)
    cm2 = np.full((128, nk), -1, dtype=np.int64)
    for r0, P, C, W in RECTS:
        for i in range(32):
            r = r0 + i
            toff = 256 * r - r * (r - 1) // 2
            cm2[P + i, C + i:C + W] = cfg.tri123 + toff + np.arange(W - i)
    tri_slots = np.zeros((128, nk), dtype=bool)
    tri_slots[0:64, :] = True
    tri_slots[64:128, 576:] = True
    idx = np.flatnonzero(tri_slots.reshape(-1))[:cfg.tri123]
    cm2.reshape(-1)[idx] = np.arange(cfg.tri123)
    colmap = cm2.reshape(-1)
    # quantize Wh to fp8e3 once, zeros on padding slots
    Whq = _to_f8(Wh, SHs)                                     # [1792, 76128]
    Whp = np.concatenate(
        [Whq, np.zeros((cfg.hidden, 1), dtype=F8)], axis=1)
    Whg = Whp[:, np.where(colmap < 0, Wh.shape[1], colmap)]   # [1792, 80896]

    Wfq = _to_f8(Wf, SFs)                                     # [14, 36864, 128]

    in_maps = []
    for c in range(cfg.n_cores):
        whr = np.zeros((rows, Whg.shape[1]), dtype=F8)
        bhr = np.zeros((1, rows), dtype=f32)
        wfr = np.zeros((cfg.slots, cfg.d2, cfg.hid), dtype=F8)
        for s, h in enumerate(HEAD_MAP[c]):
            if h < 0:
                continue
            whr[s * hid:(s + 1) * hid] = Whg[h * hid:(h + 1) * hid]
            bhr[0, s * hid:(s + 1) * hid] = bh[h * hid:(h + 1) * hid]
            wfr[s] = Wfq[h]
        whc = (whr.reshape(rows, 128, nk).transpose(1, 2, 0)[:, PERM, :]
               .reshape(128, nk * rows))
        wfc = (wfr.reshape(cfg.slots, cfg.nblk, 128, cfg.hid)
               .transpose(3, 0, 1, 2)
               .reshape(128, t2 * 128))
        in_maps.append({
            "g4": g4, "img": img,
            "bh_c": np.ascontiguousarray(
                bhr.reshape(cfg.slots, hid).T),
            "wh": np.ascontiguousarray(whc),
            "wf": np.ascontiguousarray(wfc),
        })
    return in_maps


def unshard_output(cfg: Cfg, outs, bf):
    """outs: per-core [128, t2] f32 -> [nl, 1, d2] (+ bf)."""
    res = np.empty((cfg.nl, cfg.d2), dtype=np.float32)
    for c in range(cfg.n_cores):
        # [128, t2] -> [t2, 128] -> [slots, d2]
        r = outs[c].T.reshape(cfg.slots, cfg.d2)
        for s, h in enumerate(HEAD_MAP[c]):
            if h >= 0:
                res[h] = r[s]
    res = res + bf.reshape(cfg.nl, cfg.d2)
    return np.ascontiguousarray(res[:, None, :], dtype=np.float32)


_NC_CACHE = {}


def _get_nc(cfg: Cfg):
    if cfg not in _NC_CACHE:
        _NC_CACHE[cfg] = build_nc(cfg)
    return _NC_CACHE[cfg]


def kernel(g1, g2, g3, g4, Wh, bh, Wf, bf):
    from concourse import bass_utils

    cfg = FULL
    nc = _get_nc(cfg)
    in_maps = shard_inputs(cfg, g1, g2, g3, g4, Wh, bh, Wf, bf)
    res = bass_utils.run_bass_kernel_spmd(
        nc, in_maps, core_ids=list(range(cfg.n_cores)))
    return unshard_output(cfg, [res.results[c]["out"]
                                for c in range(cfg.n_cores)], bf)
